# revision 20
# baseline (speedup 1.0000x reference)
"""Trainium2 Bass kernel for DecoupledRadialAngularLoss.

Strategy (vocab-parallel over 8 NeuronCores, like fused-linear-CE):
  - V=50257 padded to 51200 = 8*6400; core k owns vocab slice [k*6400,(k+1)*6400)
    (zero-padded W columns / p=1.0 padded teacher entries; exact host-side
    correction of the padded exp contribution).
  - Each core:
      * normalizes its W_vocab shard spatial rows (norms via ACT Square+accum
        in natural layout, rsqrt, broadcast, in-place column scale of the
        feature-major bf16 copy used by the PE),
      * computes student spatial norms the same way -> per-token inv_s,
      * GEMM G[t,v] = sp_s . u_w (bf16, fp32 PSUM accumulation),
      * ACT: exp(G*inv_s - 1) with fused per-row accumulation -> partial Z
        (cos<=1 so the fixed shift 1.0 replaces the softmax max pass),
      * ACT: log(p); DVE tensor_tensor_reduce: partial sum p*log(p) and
        partial sum p*G per row.
  - Host combines per-core row partials: logZ = 1 + log(sum_k Z_k),
    KL_row = A - inv_s*B + logZ; radial loss terms are O(B*L) and computed
    on host from the raw fp32 inputs.
"""

import math

import ml_dtypes
import numpy as np

import concourse.bass as bass
import concourse.mybir as mybir
import concourse.tile as tile
from concourse import bacc
from concourse import bass_utils

# ---- problem constants (hardcoded per contest contract) ----
B, L, N_FEAT = 2, 1024, 768
V = 50257
R_MAX = 3.0
LAMBDA_RADIAL = 0.1
T_TEMP = 1.0
LOG_V = math.log(V)
EPS = 1e-12

N_CORES = 8
VP = 6400                 # per-core padded vocab shard (50*128, 12.5*512)
V_PAD_TOTAL = N_CORES * VP  # 51200
N_PAD_LAST = V_PAD_TOTAL - V  # 943 zero-W / one-p padded columns on core 7

NT = (B * L) // 128       # 16 token tiles of 128
NF = N_FEAT // 128        # 6 feature tiles of 128
SCS = [(0, 2048), (2048, 2048), (4096, 2048), (6144, 256)]  # superchunks of VP
NSC = len(SCS)

BF16 = mybir.dt.bfloat16
F32 = mybir.dt.float32
AF = mybir.ActivationFunctionType
ALU = mybir.AluOpType

_CACHE = {}


def _patch_act_tables():
    """Make Exp and Ln resolve to the one table set containing both
    (natural_log_exp_and_others) so the kernel's alternating exp/ln
    activations don't thrash ACT table loads (~1.3us each)."""
    if _CACHE.get("act_patched"):
        return
    from concourse import bacc as bacc_mod
    orig = bacc_mod.get_activation_tables

    def patched(arch):
        tabs = {k: set(v) for k, v in orig(arch).items()}
        for name in ("exp_and_others", "exp_and_friends"):
            if name in tabs:
                tabs[name].discard(AF.Exp)
        if "natural_log" in tabs:
            tabs["natural_log"].discard(AF.Ln)
        return tabs

    bacc_mod.get_activation_tables = patched
    _CACHE["act_patched"] = True


def _build_program(stage="full"):
    """Build + compile the single-core SPMD Bass program (same NEFF, 8 cores).

    stage: debug knob - "prep" builds only the norm/scale prep, "mm" adds
    the matmuls, "nodve" adds ACT exp/ln, "full" is everything.
    """
    _patch_act_tables()
    nc = bacc.Bacc("TRN2", target_bir_lowering=False, debug=False)

    TOK = B * L
    hT_d = nc.dram_tensor("hT", (N_FEAT, TOK), BF16, kind="ExternalInput").ap()
    wT_d = nc.dram_tensor("wT", (N_FEAT, VP), BF16, kind="ExternalInput").ap()
    p_d = nc.dram_tensor("p", (TOK, VP), BF16, kind="ExternalInput").ap()

    z_d = nc.dram_tensor("Z", (128, NT), F32, kind="ExternalOutput").ap()
    a_d = nc.dram_tensor("A", (128, NT), F32, kind="ExternalOutput").ap()
    b_d = nc.dram_tensor("Bt", (128, NT), F32, kind="ExternalOutput").ap()
    is_d = nc.dram_tensor("IS", (128, NT), F32, kind="ExternalOutput").ap()

    NWV = VP // 128  # 50 vocab norm tiles

    with tile.TileContext(nc) as tc:
        with (
            tc.tile_pool(name="persist", bufs=1) as persist,
            tc.tile_pool(name="dram", bufs=1, space="DRAM") as dram,
        ):
            # ---------- resident tiles ----------
            hT_sb = persist.tile([128, NF, TOK], BF16)
            wT_sb = persist.tile([128, NF, VP], BF16)
            invw16 = persist.tile([128, VP], BF16)
            inv_s = persist.tile([128, NT], F32)
            neg1 = persist.tile([128, 1], F32)
            epsb = persist.tile([128, 1], F32)
            zparts = persist.tile([128, NT * NSC], F32)
            aparts = persist.tile([128, NT * NSC], F32)
            bparts = persist.tile([128, NT * NSC], F32)

            nc.vector.memset(neg1, -1.0)
            nc.vector.memset(epsb, 1e-30)

            with (
                tc.tile_pool(name="prep", bufs=2) as prep,
                tc.tile_pool(name="stream", bufs=3) as stream,
                tc.tile_pool(name="scratch", bufs=2) as scratch,
                tc.tile_pool(name="psum", bufs=2, space="PSUM") as psum,
            ):
                ones16 = persist.tile([128, 1], BF16)
                nc.vector.memset(ones16, 1.0)

                # GEMM operand loads on the sync (HWDGE) queue; wT arrives
                # one superchunk of columns at a time so its norm/scale
                # chains (and then the first matmuls) unblock early.
                nc.sync.dma_start(out=hT_sb,
                                  in_=hT_d.rearrange("(f pp) t -> pp f t", pp=128))
                wT_r = wT_d.rearrange("(f pp) v -> pp f v", pp=128)
                for (off, scw) in SCS:
                    nc.sync.dma_start(out=wT_sb[:, :, off:off + scw],
                                      in_=wT_r[:, :, off:off + scw])

                # ---- student norms from hT (PE ones-matmul over partitions),
                #      rsqrt done partition-major after a DRAM roundtrip ----
                hscr = dram.tile([1, TOK], F32)
                ns2h_ps = psum.tile([128, 2048], F32, tag="G")
                for f in range(NF):
                    sqh = prep.tile([128, 2048], BF16, tag="sq")
                    nc.scalar.activation(out=sqh[:, :TOK], in_=hT_sb[:, f, :],
                                         func=AF.Square)
                    for c in range(0, TOK, 512):
                        nc.tensor.matmul(ns2h_ps[0:1, c:c + 512], ones16,
                                         sqh[:, c:c + 512],
                                         start=(f == 0), stop=(f == NF - 1))
                ns2h_fm = prep.tile([1, 2048], F32, tag="nsfm")
                nc.vector.tensor_copy(out=ns2h_fm[:, :TOK], in_=ns2h_ps[0:1, :TOK])
                nc.sync.dma_start(out=hscr, in_=ns2h_fm[:, :TOK])
                ns2_s = persist.tile([128, NT], F32)
                nc.sync.dma_start(out=ns2_s,
                                  in_=hscr.rearrange("one (j pp) -> one pp j", pp=128)[0])
                nc.scalar.activation(out=inv_s, in_=ns2_s, func=AF.Sqrt, bias=epsb)
                nc.vector.reciprocal(out=inv_s, in_=inv_s)
                nc.sync.dma_start(out=is_d, in_=inv_s)

                # ---- vocab norms + rsqrt + column scale, per superchunk ----
                wscr = dram.tile([1, VP], F32)      # flat ns2_w
                wscr16 = dram.tile([1, VP], BF16)   # flat inv_w (bf16)
                wscr_pm = wscr.rearrange("one (i pp) -> one pp i", pp=128)[0]
                wscr16_pm = wscr16.rearrange("one (i pp) -> one pp i", pp=128)[0]
                for (off, scw) in SCS:
                    ni = scw // 128
                    i0 = off // 128
                    ns2w_ps = psum.tile([128, 2048], F32, tag="G")
                    for f in range(NF):
                        sqw = prep.tile([128, 2048], BF16, tag="sq")
                        nc.scalar.activation(out=sqw[:, :scw],
                                             in_=wT_sb[:, f, off:off + scw],
                                             func=AF.Square)
                        for c in range(0, scw, 512):
                            cw = min(512, scw - c)
                            nc.tensor.matmul(ns2w_ps[0:1, c:c + cw], ones16,
                                             sqw[:, c:c + cw],
                                             start=(f == 0), stop=(f == NF - 1))
                    ns2w_fm = prep.tile([1, 2048], F32, tag="nsfm")
                    nc.vector.tensor_copy(out=ns2w_fm[:, :scw], in_=ns2w_ps[0:1, :scw])
                    nc.sync.dma_start(out=wscr[:, off:off + scw], in_=ns2w_fm[:, :scw])
                    ns2w_pm = prep.tile([128, NWV], F32, tag="ns2pm")
                    nc.sync.dma_start(out=ns2w_pm[:, :ni], in_=wscr_pm[:, i0:i0 + ni])
                    nc.scalar.activation(out=ns2w_pm[:, :ni], in_=ns2w_pm[:, :ni],
                                         func=AF.Sqrt, bias=epsb)
                    nc.vector.reciprocal(out=ns2w_pm[:, :ni], in_=ns2w_pm[:, :ni])
                    invw_pm16 = prep.tile([128, NWV], BF16, tag="invwpm")
                    nc.vector.tensor_copy(out=invw_pm16[:, :ni], in_=ns2w_pm[:, :ni])
                    nc.sync.dma_start(out=wscr16_pm[:, i0:i0 + ni],
                                      in_=invw_pm16[:, :ni])
                    sl16 = wscr16[0:1, off:off + scw]
                    bc_src = bass.AP(tensor=sl16.tensor, offset=sl16.offset,
                                     ap=[[0, 128], [1, scw]])
                    nc.sync.dma_start(out=invw16[:, off:off + scw], in_=bc_src)
                    # in-place column scale of this superchunk
                    sl = invw16[:, off:off + scw]
                    invw_b = bass.AP(tensor=sl.tensor, offset=sl.offset,
                                     ap=[sl.ap[0], [0, NF], sl.ap[1]])
                    nc.vector.tensor_tensor(out=wT_sb[:, :, off:off + scw],
                                            in0=wT_sb[:, :, off:off + scw],
                                            in1=invw_b, op=ALU.mult)

                # ---------- main loop ----------
                for j in range(NT if stage != "prep" else 0):
                    for s, (off, scw) in enumerate(SCS):
                        k = j * NSC + s
                        G = psum.tile([128, 2048], F32, tag="G")
                        for f in range(NF):
                            for c in range(0, scw, 512):
                                cw = min(512, scw - c)
                                nc.tensor.matmul(
                                    G[:, c:c + cw],
                                    hT_sb[:, f, j * 128:(j + 1) * 128],
                                    wT_sb[:, f, off + c:off + c + cw],
                                    start=(f == 0), stop=(f == NF - 1),
                                )
                        if stage == "mm":
                            continue
                        p_sb = stream.tile([128, 2048], BF16, tag="p")
                        nc.sync.dma_start(out=p_sb[:, :scw],
                                          in_=p_d[j * 128:(j + 1) * 128, off:off + scw])

                        scr_e = scratch.tile([128, 2048], BF16, tag="scre")
                        nc.scalar.activation(out=scr_e[:, :scw], in_=G[:, :scw],
                                             func=AF.Exp, bias=neg1,
                                             scale=inv_s[:, j:j + 1],
                                             accum_out=zparts[:, k:k + 1])

                        logp = scratch.tile([128, 2048], F32, tag="logp")
                        nc.scalar.activation(out=logp[:, :scw], in_=p_sb[:, :scw],
                                             func=AF.Ln)
                        if stage == "nodve":
                            continue

                        if stage in ("full", "ttra"):
                            # aparts = sum_v p*log(p)
                            scr_a = scratch.tile([128, 2048], BF16, tag="scra")
                            nc.vector.affine_mul_reduce(
                                out=scr_a[:, :scw], accum_out=aparts[:, k:k + 1],
                                in0=p_sb[:, :scw], in1=logp[:, :scw],
                                scale=1.0, bias=0.0)

                        if stage in ("full", "ttrb"):
                            # bparts = sum_v (G*inv_s)*p = sum_v p*l
                            scr_b = scratch.tile([128, 2048], BF16, tag="scrb")
                            nc.vector.affine_mul_reduce(
                                out=scr_b[:, :scw], accum_out=bparts[:, k:k + 1],
                                in0=G[:, :scw], in1=p_sb[:, :scw],
                                scale=inv_s[:, j:j + 1], bias=0.0)

                # collapse superchunk partials and write out
                if stage == "full":
                    zpm = persist.tile([128, NT], F32)
                    apm = persist.tile([128, NT], F32)
                    bpm = persist.tile([128, NT], F32)
                    for (src, dst) in ((zparts, zpm), (aparts, apm), (bparts, bpm)):
                        nc.vector.tensor_reduce(
                            out=dst, in_=src.rearrange("pp (j s) -> pp j s", s=NSC),
                            axis=mybir.AxisListType.X, op=ALU.add)
                    nc.sync.dma_start(out=z_d, in_=zpm)
                    nc.sync.dma_start(out=a_d, in_=apm)
                    nc.sync.dma_start(out=b_d, in_=bpm)

    nc.compile()
    return nc


def _get_program():
    if "nc" not in _CACHE:
        _CACHE["nc"] = _build_program()
    return _CACHE["nc"]


def _prep_inputs(h_student, W_vocab, p_teacher):
    """Host-side shard/layout prep (numpy only)."""
    TOK = B * L
    sp_s = np.ascontiguousarray(h_student.reshape(TOK, N_FEAT + 1)[:, 1:])
    sp_w = W_vocab[:, 1:]

    hT16 = np.ascontiguousarray(sp_s.astype(ml_dtypes.bfloat16).T)

    # padded W: zeros beyond V
    wn16_full = np.zeros((V_PAD_TOTAL, N_FEAT), dtype=ml_dtypes.bfloat16)
    wn16_full[:V] = sp_w.astype(ml_dtypes.bfloat16)
    wT16_full = np.ascontiguousarray(wn16_full.T)

    # padded p: ones beyond V (log 1 = 0, and padded G columns are 0)
    p16_full = np.ones((TOK, V_PAD_TOTAL), dtype=ml_dtypes.bfloat16)
    p16_full[:, :V] = p_teacher.reshape(TOK, V).astype(ml_dtypes.bfloat16)

    in_maps = []
    for k in range(N_CORES):
        lo, hi = k * VP, (k + 1) * VP
        in_maps.append({
            "hT": hT16,
            "wT": np.ascontiguousarray(wT16_full[:, lo:hi]),
            "p": np.ascontiguousarray(p16_full[:, lo:hi]),
        })
    return in_maps


def _combine(results, h_student, teacher_entropy):
    """Host-side gather of per-core row partials + tiny radial part."""
    TOK = B * L

    def pm_to_tok(arr):  # [128, NT] partition-major -> [TOK] token order
        return np.ascontiguousarray(arr.T).reshape(TOK)

    Z = np.zeros(TOK, np.float64)
    A = np.zeros(TOK, np.float64)
    Bp = np.zeros(TOK, np.float64)
    for k in range(N_CORES):
        Z += pm_to_tok(results[k]["Z"]).astype(np.float64)
        A += pm_to_tok(results[k]["A"]).astype(np.float64)
        Bp += pm_to_tok(results[k]["Bt"]).astype(np.float64)
    inv_s = pm_to_tok(results[0]["IS"]).astype(np.float64)

    # remove the padded columns' exp(0*inv_s - 1) contribution (core 7)
    Z -= N_PAD_LAST * math.exp(-1.0)

    logZ = 1.0 + np.log(Z)
    kl_rows = A - Bp + logZ  # Bp already inv_s-scaled on device
    kl = kl_rows.sum() / TOK
    l_angular = kl * (T_TEMP ** 2)

    # radial part from the raw fp32 inputs (O(B*L) work)
    x0 = np.clip(h_student.reshape(TOK, N_FEAT + 1)[:, 0].astype(np.float64),
                 1.0 + 1e-7, None)
    r_s = np.arccosh(x0)
    H_norm = np.clip(teacher_entropy.reshape(TOK).astype(np.float64) / LOG_V, 0.0, 1.0)
    r_target = (1.0 / (1.0 + np.exp(H_norm))) * R_MAX  # sigmoid(-H) * R_MAX
    l_radial = np.mean((r_s - r_target) ** 2)
    l_total = l_angular + LAMBDA_RADIAL * l_radial

    return np.array([l_total, l_angular, l_radial,
                     r_s.mean(), r_target.mean(), H_norm.mean()], dtype=np.float32)


def kernel(h_student, W_vocab, p_teacher, teacher_entropy):
    nc = _get_program()
    in_maps = _prep_inputs(h_student, W_vocab, p_teacher)
    res = bass_utils.run_bass_kernel_spmd(nc, in_maps, core_ids=list(range(N_CORES)))
    return _combine(res.results, h_student, teacher_entropy)


# revision 40
# speedup vs baseline: 2.2728x; 2.2728x over previous
"""Trainium2 Bass kernel for DecoupledRadialAngularLoss.

Strategy (vocab-parallel over 8 NeuronCores, like fused-linear-CE):
  - V=50257 padded to 51200 = 8*6400; core k owns vocab slice [k*6400,(k+1)*6400)
    (zero-padded W columns / p=1.0 padded teacher entries; exact host-side
    correction of the padded exp contribution).
  - Each core:
      * normalizes its W_vocab shard spatial rows (norms via ACT Square+accum
        in natural layout, rsqrt, broadcast, in-place column scale of the
        feature-major bf16 copy used by the PE),
      * computes student spatial norms the same way -> per-token inv_s,
      * GEMM G[t,v] = sp_s . u_w (bf16, fp32 PSUM accumulation),
      * ACT: exp(G*inv_s - 1) with fused per-row accumulation -> partial Z
        (cos<=1 so the fixed shift 1.0 replaces the softmax max pass),
      * ACT: log(p); DVE tensor_tensor_reduce: partial sum p*log(p) and
        partial sum p*G per row.
  - Host combines per-core row partials: logZ = 1 + log(sum_k Z_k),
    KL_row = A - inv_s*B + logZ; radial loss terms are O(B*L) and computed
    on host from the raw fp32 inputs.
"""

import math

import ml_dtypes
import numpy as np

import concourse.bass as bass
import concourse.mybir as mybir
import concourse.tile as tile
from concourse import bacc
from concourse import bass_utils

# ---- problem constants (hardcoded per contest contract) ----
B, L, N_FEAT = 2, 1024, 768
V = 50257
R_MAX = 3.0
LAMBDA_RADIAL = 0.1
T_TEMP = 1.0
LOG_V = math.log(V)
EPS = 1e-12

N_CORES = 8
VP = 6400                 # per-core padded vocab shard (50*128, 12.5*512)
V_PAD_TOTAL = N_CORES * VP  # 51200
N_PAD_LAST = V_PAD_TOTAL - V  # 943 zero-W / one-p padded columns on core 7

NT = (B * L) // 128       # 16 token tiles of 128
NF = N_FEAT // 128        # 6 feature tiles of 128
SCW = 1024                # superchunk width (2 PSUM banks; 4-deep pipeline)
SCS = [(o, min(SCW, VP - o)) for o in range(0, VP, SCW)]  # 6x1024 + 256
NSC = len(SCS)

BF16 = mybir.dt.bfloat16
FP8 = mybir.dt.float8e4
F32 = mybir.dt.float32
AF = mybir.ActivationFunctionType
ALU = mybir.AluOpType
NF2 = NF // 2             # 3 DoubleRow k-tile pairs

_CACHE = {}


def _patch_act_tables():
    """Make Exp and Ln resolve to the one table set containing both
    (natural_log_exp_and_others) so the kernel's alternating exp/ln
    activations don't thrash ACT table loads (~1.3us each)."""
    if _CACHE.get("act_patched"):
        return
    from concourse import bacc as bacc_mod
    orig = bacc_mod.get_activation_tables

    def patched(arch):
        tabs = {k: set(v) for k, v in orig(arch).items()}
        for name in ("exp_and_others", "exp_and_friends"):
            if name in tabs:
                tabs[name].discard(AF.Exp)
        if "natural_log" in tabs:
            tabs["natural_log"].discard(AF.Ln)
        return tabs

    bacc_mod.get_activation_tables = patched
    _CACHE["act_patched"] = True


def _build_program(stage="full"):
    """Build + compile the single-core SPMD Bass program (same NEFF, 8 cores).

    stage: debug knob - "prep" builds only the norm/scale prep, "mm" adds
    the matmuls, "nodve" adds ACT exp/ln, "full" is everything.
    """
    _patch_act_tables()
    nc = bacc.Bacc("TRN2", target_bir_lowering=False, debug=False)

    TOK = B * L
    hT_d = nc.dram_tensor("hT", (N_FEAT, TOK), FP8, kind="ExternalInput").ap()
    wT_d = nc.dram_tensor("wT", (N_FEAT, VP), BF16, kind="ExternalInput").ap()
    p_d = nc.dram_tensor("p", (TOK, VP), BF16, kind="ExternalInput").ap()

    z_d = nc.dram_tensor("Z", (128, NT), F32, kind="ExternalOutput").ap()
    a_d = nc.dram_tensor("A", (128, NT), F32, kind="ExternalOutput").ap()
    b_d = nc.dram_tensor("Bt", (128, NT), F32, kind="ExternalOutput").ap()
    is_d = nc.dram_tensor("IS", (128, NT), F32, kind="ExternalOutput").ap()

    NWV = VP // 128  # 50 vocab norm tiles

    with tile.TileContext(nc) as tc:
        with (
            tc.tile_pool(name="persist", bufs=1) as persist,
            tc.tile_pool(name="dram", bufs=1, space="DRAM") as dram,
        ):
            # ---------- resident tiles ----------
            # fp8 GEMM operands in DoubleRow layout [k, t, q, x]
            # (feature f = (t*2+q)*128 + k)
            hT_sb = persist.tile([128, NF2, 2, TOK], FP8)
            wT_sb = persist.tile([128, NF2, 2, VP], FP8)
            invw16 = persist.tile([128, VP], BF16)
            inv_s = persist.tile([128, NT], F32)
            neg1 = persist.tile([128, 1], F32)
            epsb = persist.tile([128, 1], F32)
            zparts = persist.tile([128, NT * NSC], F32)
            aparts = persist.tile([128, NT * NSC], F32)
            bparts = persist.tile([128, NT * NSC], F32)

            nc.vector.memset(neg1, -1.0)
            nc.vector.memset(epsb, 1e-30)
            # aparts only uses slot 0 of each t-tile group; zero the rest
            nc.vector.memset(aparts, 0.0)

            with (
                tc.tile_pool(name="prep", bufs=2) as prep,
                tc.tile_pool(name="stream", bufs=3) as stream,
                tc.tile_pool(name="scratch", bufs=2) as scratch,
                tc.tile_pool(name="psum", bufs=4, space="PSUM") as psum,
            ):
                ones16 = persist.tile([128, 1], BF16)
                nc.vector.memset(ones16, 1.0)

                # fp8 student operand load (DoubleRow view of natural layout)
                nc.sync.dma_start(
                    out=hT_sb,
                    in_=hT_d.rearrange("(t q pp) x -> pp t q x", pp=128, q=2))

                # ---- student norms from hT8 (PE ones-matmul over partitions),
                #      rsqrt done partition-major after a DRAM roundtrip ----
                hscr = dram.tile([1, TOK], F32)
                for hc in range(0, TOK, SCW):
                    ns2h_ps = psum.tile([128, SCW], F32, tag="G")
                    for fq in range(NF):
                        sqh = prep.tile([128, SCW], BF16, tag="sq")
                        nc.scalar.activation(
                            out=sqh, in_=hT_sb[:, fq // 2, fq % 2, hc:hc + SCW],
                            func=AF.Square)
                        for c in range(0, SCW, 512):
                            nc.tensor.matmul(ns2h_ps[0:1, c:c + 512], ones16,
                                             sqh[:, c:c + 512],
                                             start=(fq == 0), stop=(fq == NF - 1))
                    ns2h_fm = prep.tile([1, SCW], F32, tag="nsfm")
                    nc.vector.tensor_copy(out=ns2h_fm, in_=ns2h_ps[0:1, :])
                    nc.sync.dma_start(out=hscr[:, hc:hc + SCW], in_=ns2h_fm)
                # rsqrt = exp(-0.5*ln(x)) - stays in the natural_log_exp
                # table set (no Sqrt set load/thrash)
                ns2_s = persist.tile([128, NT], F32)
                nc.sync.dma_start(out=ns2_s,
                                  in_=hscr.rearrange("one (j pp) -> one pp j", pp=128)[0])
                nc.scalar.activation(out=ns2_s, in_=ns2_s, func=AF.Ln, bias=epsb)
                nc.scalar.activation(out=inv_s, in_=ns2_s, func=AF.Exp, scale=-0.5)
                nc.sync.dma_start(out=is_d, in_=inv_s)

                # ---- vocab norms + rsqrt + column scale, per superchunk.
                # wT arrives as transient bf16 slabs; squares and the
                # normalize run at bf16 DVE rates, the normalized result is
                # written straight into the fp8 DoubleRow-layout operand. ----
                wscr16 = dram.tile([1, VP], BF16)   # flat inv_w (bf16)
                wT_r = wT_d.rearrange("(f pp) v -> pp f v", pp=128)
                # [128, (t q), v] view of the fp8 operand; f-tile = t*2+q
                wT8v = wT_sb.rearrange("pp t q v -> pp (t q) v")
                for (off, scw) in SCS:
                    slab = prep.tile([128, NF, SCW], BF16, tag="slab")
                    nc.sync.dma_start(out=slab[:, :, :scw],
                                      in_=wT_r[:, :, off:off + scw])
                    ns2w_ps = psum.tile([128, SCW], F32, tag="G")
                    for f in range(NF):
                        sqw = prep.tile([128, SCW], BF16, tag="sq")
                        # split squares across DVE and ACT to balance load
                        if f % 2 == 0:
                            nc.vector.tensor_tensor(out=sqw[:, :scw],
                                                    in0=slab[:, f, :scw],
                                                    in1=slab[:, f, :scw],
                                                    op=ALU.mult)
                        else:
                            nc.scalar.activation(out=sqw[:, :scw],
                                                 in_=slab[:, f, :scw],
                                                 func=AF.Square)
                        for c in range(0, scw, 512):
                            cw = min(512, scw - c)
                            nc.tensor.matmul(ns2w_ps[0:1, c:c + cw], ones16,
                                             sqw[:, c:c + cw],
                                             start=(f == 0), stop=(f == NF - 1))
                    # rsqrt free-major on one partition: exp(-0.5*ln(ns2+eps))
                    ns2w_fm = prep.tile([1, SCW], F32, tag="nsfm")
                    nc.scalar.activation(out=ns2w_fm[:, :scw], in_=ns2w_ps[0:1, :scw],
                                         func=AF.Ln, bias=epsb[0:1])
                    invw_fm16 = prep.tile([1, SCW], BF16, tag="invwfm")
                    nc.scalar.activation(out=invw_fm16[:, :scw], in_=ns2w_fm[:, :scw],
                                         func=AF.Exp, scale=-0.5)
                    nc.sync.dma_start(out=wscr16[:, off:off + scw],
                                      in_=invw_fm16[:, :scw])
                    sl16 = wscr16[0:1, off:off + scw]
                    bc_src = bass.AP(tensor=sl16.tensor, offset=sl16.offset,
                                     ap=[[0, 128], [1, scw]])
                    nc.sync.dma_start(out=invw16[:, off:off + scw], in_=bc_src)
                    # normalize + cast into the fp8 operand
                    sl = invw16[:, off:off + scw]
                    invw_b = bass.AP(tensor=sl.tensor, offset=sl.offset,
                                     ap=[sl.ap[0], [0, NF], sl.ap[1]])
                    nc.vector.tensor_tensor(out=wT8v[:, :, off:off + scw],
                                            in0=slab[:, :, :scw],
                                            in1=invw_b, op=ALU.mult)

                # ---------- main loop ----------
                # Per token-tile: one full-row p load + ln + p*log(p) reduce
                # (independent of the GEMM), and per 1024-wide superchunk the
                # G matmuls + fused exp/accum + p*l reduce (PSUM-gated, 4-deep).
                for j in range(NT if stage != "prep" else 0):
                    if stage != "mm":
                        p_sb = stream.tile([128, VP], BF16, tag="p", bufs=2)
                        nc.sync.dma_start(out=p_sb,
                                          in_=p_d[j * 128:(j + 1) * 128, :])

                    for s, (off, scw) in enumerate(SCS):
                        k = j * NSC + s
                        G = psum.tile([128, SCW], F32, tag="G")
                        for t in range(NF2):
                            for c in range(0, scw, 512):
                                cw = min(512, scw - c)
                                nc.tensor.matmul(
                                    G[:, c:c + cw],
                                    hT_sb[:, t, :, j * 128:(j + 1) * 128],
                                    wT_sb[:, t, :, off + c:off + c + cw],
                                    start=(t == 0), stop=(t == NF2 - 1),
                                    perf_mode=mybir.MatmulPerfMode.DoubleRow,
                                )
                        if stage == "mm":
                            continue

                        scr_e = scratch.tile([128, SCW], BF16, tag="scre")
                        nc.scalar.activation(out=scr_e[:, :scw], in_=G[:, :scw],
                                             func=AF.Exp, bias=neg1,
                                             scale=inv_s[:, j:j + 1],
                                             accum_out=zparts[:, k:k + 1])
                        if stage == "nodve":
                            continue

                        # bparts = sum_v (G*inv_s)*p = sum_v p*l
                        scr_b = scratch.tile([128, SCW], BF16, tag="scrb")
                        nc.vector.affine_mul_reduce(
                            out=scr_b[:, :scw], accum_out=bparts[:, k:k + 1],
                            in0=G[:, :scw], in1=p_sb[:, off:off + scw],
                            scale=inv_s[:, j:j + 1], bias=0.0)

                        # p*log(p) path is G-independent: two 3200-wide
                        # ln + reduce pairs per t-tile, spread mid-tile
                        if stage != "mm" and s in (2, 5):
                            ho = (VP // 2) * (s == 5)
                            hw = VP // 2
                            logp = scratch.tile([128, VP // 2], F32, tag="logp")
                            nc.scalar.activation(out=logp,
                                                 in_=p_sb[:, ho:ho + hw], func=AF.Ln)
                            # aparts = sum_v p*log(p); in-place overwrite of logp
                            nc.vector.affine_mul_reduce(
                                out=logp, accum_out=aparts[:, k:k + 1],
                                in0=p_sb[:, ho:ho + hw], in1=logp,
                                scale=1.0, bias=0.0)

                # collapse superchunk partials and write out
                if stage == "full":
                    zpm = persist.tile([128, NT], F32)
                    apm = persist.tile([128, NT], F32)
                    bpm = persist.tile([128, NT], F32)
                    for (src, dst) in ((zparts, zpm), (aparts, apm), (bparts, bpm)):
                        nc.vector.tensor_reduce(
                            out=dst, in_=src.rearrange("pp (j s) -> pp j s", s=NSC),
                            axis=mybir.AxisListType.X, op=ALU.add)
                    nc.sync.dma_start(out=z_d, in_=zpm)
                    nc.sync.dma_start(out=a_d, in_=apm)
                    nc.sync.dma_start(out=b_d, in_=bpm)

    nc.compile()
    return nc


def _get_program():
    if "nc" not in _CACHE:
        _CACHE["nc"] = _build_program()
    return _CACHE["nc"]


def _prep_inputs(h_student, W_vocab, p_teacher):
    """Host-side shard/layout prep (numpy only)."""
    TOK = B * L
    sp_s = np.ascontiguousarray(h_student.reshape(TOK, N_FEAT + 1)[:, 1:])
    sp_w = W_vocab[:, 1:]

    hT8 = np.ascontiguousarray(sp_s.astype(ml_dtypes.float8_e4m3).T)

    # padded W: zeros beyond V
    wn16_full = np.zeros((V_PAD_TOTAL, N_FEAT), dtype=ml_dtypes.bfloat16)
    wn16_full[:V] = sp_w.astype(ml_dtypes.bfloat16)
    wT16_full = np.ascontiguousarray(wn16_full.T)

    # padded p: ones beyond V (log 1 = 0, and padded G columns are 0)
    p16_full = np.ones((TOK, V_PAD_TOTAL), dtype=ml_dtypes.bfloat16)
    p16_full[:, :V] = p_teacher.reshape(TOK, V).astype(ml_dtypes.bfloat16)

    in_maps = []
    for k in range(N_CORES):
        lo, hi = k * VP, (k + 1) * VP
        in_maps.append({
            "hT": hT8,
            "wT": np.ascontiguousarray(wT16_full[:, lo:hi]),
            "p": np.ascontiguousarray(p16_full[:, lo:hi]),
        })
    return in_maps


def _combine(results, h_student, teacher_entropy):
    """Host-side gather of per-core row partials + tiny radial part."""
    TOK = B * L

    def pm_to_tok(arr):  # [128, NT] partition-major -> [TOK] token order
        return np.ascontiguousarray(arr.T).reshape(TOK)

    Z = np.zeros(TOK, np.float64)
    A = np.zeros(TOK, np.float64)
    Bp = np.zeros(TOK, np.float64)
    for k in range(N_CORES):
        Z += pm_to_tok(results[k]["Z"]).astype(np.float64)
        A += pm_to_tok(results[k]["A"]).astype(np.float64)
        Bp += pm_to_tok(results[k]["Bt"]).astype(np.float64)
    inv_s = pm_to_tok(results[0]["IS"]).astype(np.float64)

    # remove the padded columns' exp(0*inv_s - 1) contribution (core 7)
    Z -= N_PAD_LAST * math.exp(-1.0)

    logZ = 1.0 + np.log(Z)
    kl_rows = A - Bp + logZ  # Bp already inv_s-scaled on device
    kl = kl_rows.sum() / TOK
    l_angular = kl * (T_TEMP ** 2)

    # radial part from the raw fp32 inputs (O(B*L) work)
    x0 = np.clip(h_student.reshape(TOK, N_FEAT + 1)[:, 0].astype(np.float64),
                 1.0 + 1e-7, None)
    r_s = np.arccosh(x0)
    H_norm = np.clip(teacher_entropy.reshape(TOK).astype(np.float64) / LOG_V, 0.0, 1.0)
    r_target = (1.0 / (1.0 + np.exp(H_norm))) * R_MAX  # sigmoid(-H) * R_MAX
    l_radial = np.mean((r_s - r_target) ** 2)
    l_total = l_angular + LAMBDA_RADIAL * l_radial

    return np.array([l_total, l_angular, l_radial,
                     r_s.mean(), r_target.mean(), H_norm.mean()], dtype=np.float32)


def kernel(h_student, W_vocab, p_teacher, teacher_entropy):
    nc = _get_program()
    in_maps = _prep_inputs(h_student, W_vocab, p_teacher)
    res = bass_utils.run_bass_kernel_spmd(nc, in_maps, core_ids=list(range(N_CORES)))
    return _combine(res.results, h_student, teacher_entropy)


# revision 43
# speedup vs baseline: 2.4441x; 1.0754x over previous
"""Trainium2 Bass kernel for DecoupledRadialAngularLoss.

Strategy (vocab-parallel over 8 NeuronCores, like fused-linear-CE):
  - V=50257 padded to 51200 = 8*6400; core k owns vocab slice [k*6400,(k+1)*6400)
    (zero-padded W columns / p=1.0 padded teacher entries; exact host-side
    correction of the padded exp contribution).
  - Each core:
      * normalizes its W_vocab shard spatial rows (norms via ACT Square+accum
        in natural layout, rsqrt, broadcast, in-place column scale of the
        feature-major bf16 copy used by the PE),
      * computes student spatial norms the same way -> per-token inv_s,
      * GEMM G[t,v] = sp_s . u_w (bf16, fp32 PSUM accumulation),
      * ACT: exp(G*inv_s - 1) with fused per-row accumulation -> partial Z
        (cos<=1 so the fixed shift 1.0 replaces the softmax max pass),
      * ACT: log(p); DVE tensor_tensor_reduce: partial sum p*log(p) and
        partial sum p*G per row.
  - Host combines per-core row partials: logZ = 1 + log(sum_k Z_k),
    KL_row = A - inv_s*B + logZ; radial loss terms are O(B*L) and computed
    on host from the raw fp32 inputs.
"""

import math

import ml_dtypes
import numpy as np

import concourse.bass as bass
import concourse.mybir as mybir
import concourse.tile as tile
from concourse import bacc
from concourse import bass_utils

# ---- problem constants (hardcoded per contest contract) ----
B, L, N_FEAT = 2, 1024, 768
V = 50257
R_MAX = 3.0
LAMBDA_RADIAL = 0.1
T_TEMP = 1.0
LOG_V = math.log(V)
EPS = 1e-12

N_CORES = 8
VP = 6400                 # per-core padded vocab shard (50*128, 12.5*512)
V_PAD_TOTAL = N_CORES * VP  # 51200
N_PAD_LAST = V_PAD_TOTAL - V  # 943 zero-W / one-p padded columns on core 7

NT = (B * L) // 128       # 16 token tiles of 128
NF = N_FEAT // 128        # 6 feature tiles of 128
SCW = 1024                # superchunk width (2 PSUM banks; 4-deep pipeline)
SCS = [(o, min(SCW, VP - o)) for o in range(0, VP, SCW)]  # 6x1024 + 256
NSC = len(SCS)

BF16 = mybir.dt.bfloat16
FP8 = mybir.dt.float8e4
F32 = mybir.dt.float32
AF = mybir.ActivationFunctionType
ALU = mybir.AluOpType
NF2 = NF // 2             # 3 DoubleRow k-tile pairs

_CACHE = {}


def _patch_act_tables():
    """Make Exp and Ln resolve to the one table set containing both
    (natural_log_exp_and_others) so the kernel's alternating exp/ln
    activations don't thrash ACT table loads (~1.3us each)."""
    if _CACHE.get("act_patched"):
        return
    from concourse import bacc as bacc_mod
    orig = bacc_mod.get_activation_tables

    def patched(arch):
        tabs = {k: set(v) for k, v in orig(arch).items()}
        for name in ("exp_and_others", "exp_and_friends"):
            if name in tabs:
                tabs[name].discard(AF.Exp)
        if "natural_log" in tabs:
            tabs["natural_log"].discard(AF.Ln)
        return tabs

    bacc_mod.get_activation_tables = patched
    _CACHE["act_patched"] = True


def _build_program(stage="full"):
    """Build + compile the single-core SPMD Bass program (same NEFF, 8 cores).

    stage: debug knob - "prep" builds only the norm/scale prep, "mm" adds
    the matmuls, "nodve" adds ACT exp/ln, "full" is everything.
    """
    _patch_act_tables()
    nc = bacc.Bacc("TRN2", target_bir_lowering=False, debug=False)

    TOK = B * L
    hT_d = nc.dram_tensor("hT", (N_FEAT, TOK), FP8, kind="ExternalInput").ap()
    wT_d = nc.dram_tensor("wT", (N_FEAT, VP), BF16, kind="ExternalInput").ap()
    p_d = nc.dram_tensor("p", (TOK, VP), BF16, kind="ExternalInput").ap()

    z_d = nc.dram_tensor("Z", (128, NT), F32, kind="ExternalOutput").ap()
    a_d = nc.dram_tensor("A", (128, NT), F32, kind="ExternalOutput").ap()
    b_d = nc.dram_tensor("Bt", (128, NT), F32, kind="ExternalOutput").ap()
    is_d = nc.dram_tensor("IS", (128, NT), F32, kind="ExternalOutput").ap()

    NWV = VP // 128  # 50 vocab norm tiles

    with tile.TileContext(nc) as tc:
        with (
            tc.tile_pool(name="persist", bufs=1) as persist,
            tc.tile_pool(name="dram", bufs=1, space="DRAM") as dram,
        ):
            # ---------- resident tiles ----------
            # fp8 GEMM operands in DoubleRow layout [k, t, q, x]
            # (feature f = (t*2+q)*128 + k)
            hT_sb = persist.tile([128, NF2, 2, TOK], FP8)
            wT_sb = persist.tile([128, NF2, 2, VP], FP8)
            invw16 = persist.tile([128, VP], BF16)
            inv_s = persist.tile([128, NT], F32)
            neg1 = persist.tile([128, 1], F32)
            epsb = persist.tile([128, 1], F32)
            zparts = persist.tile([128, NT * NSC], F32)
            aparts = persist.tile([128, NT * NSC], F32)
            bparts = persist.tile([128, NT * NSC], F32)

            nc.vector.memset(neg1, -1.0)
            nc.vector.memset(epsb, 1e-30)
            # aparts only uses slot 0 of each t-tile group; zero the rest
            nc.vector.memset(aparts, 0.0)

            with (
                tc.tile_pool(name="prep", bufs=2) as prep,
                tc.tile_pool(name="stream", bufs=3) as stream,
                tc.tile_pool(name="scratch", bufs=2) as scratch,
                tc.tile_pool(name="psum", bufs=4, space="PSUM") as psum,
            ):
                ones16 = persist.tile([128, 1], BF16)
                nc.vector.memset(ones16, 1.0)

                # fp8 student operand load (DoubleRow view of natural layout)
                nc.sync.dma_start(
                    out=hT_sb,
                    in_=hT_d.rearrange("(t q pp) x -> pp t q x", pp=128, q=2))

                # ---- student norms from hT8 (PE ones-matmul over partitions),
                #      rsqrt done partition-major after a DRAM roundtrip ----
                hscr = dram.tile([1, TOK], F32)
                for hc in range(0, TOK, SCW):
                    ns2h_ps = psum.tile([128, SCW], F32, tag="G")
                    for fq in range(NF):
                        sqh = prep.tile([128, SCW], BF16, tag="sq")
                        nc.scalar.activation(
                            out=sqh, in_=hT_sb[:, fq // 2, fq % 2, hc:hc + SCW],
                            func=AF.Square)
                        for c in range(0, SCW, 512):
                            nc.tensor.matmul(ns2h_ps[0:1, c:c + 512], ones16,
                                             sqh[:, c:c + 512],
                                             start=(fq == 0), stop=(fq == NF - 1))
                    ns2h_fm = prep.tile([1, SCW], F32, tag="nsfm")
                    nc.vector.tensor_copy(out=ns2h_fm, in_=ns2h_ps[0:1, :])
                    nc.sync.dma_start(out=hscr[:, hc:hc + SCW], in_=ns2h_fm)
                # rsqrt = exp(-0.5*ln(x)) - stays in the natural_log_exp
                # table set (no Sqrt set load/thrash)
                ns2_s = persist.tile([128, NT], F32)
                nc.sync.dma_start(out=ns2_s,
                                  in_=hscr.rearrange("one (j pp) -> one pp j", pp=128)[0])
                nc.scalar.activation(out=ns2_s, in_=ns2_s, func=AF.Ln, bias=epsb)
                nc.scalar.activation(out=inv_s, in_=ns2_s, func=AF.Exp, scale=-0.5)
                nc.sync.dma_start(out=is_d, in_=inv_s)

                # ---- vocab norms + rsqrt + column scale, per superchunk.
                # wT arrives as transient bf16 slabs; squares and the
                # normalize run at bf16 DVE rates, the normalized result is
                # written straight into the fp8 DoubleRow-layout operand. ----
                wscr16 = dram.tile([1, VP], BF16)   # flat inv_w (bf16)
                wT_r = wT_d.rearrange("(f pp) v -> pp f v", pp=128)
                # [128, (t q), v] view of the fp8 operand; f-tile = t*2+q
                wT8v = wT_sb.rearrange("pp t q v -> pp (t q) v")
                for (off, scw) in SCS:
                    slab = prep.tile([128, NF, SCW], BF16, tag="slab", bufs=3)
                    nc.sync.dma_start(out=slab[:, :, :scw],
                                      in_=wT_r[:, :, off:off + scw])
                    ns2w_ps = psum.tile([128, SCW], F32, tag="G")
                    for f in range(NF):
                        sqw = prep.tile([128, SCW], BF16, tag="sq")
                        # split squares across DVE and ACT to balance load
                        if f % 2 == 0:
                            nc.vector.tensor_tensor(out=sqw[:, :scw],
                                                    in0=slab[:, f, :scw],
                                                    in1=slab[:, f, :scw],
                                                    op=ALU.mult)
                        else:
                            nc.scalar.activation(out=sqw[:, :scw],
                                                 in_=slab[:, f, :scw],
                                                 func=AF.Square)
                        for c in range(0, scw, 512):
                            cw = min(512, scw - c)
                            nc.tensor.matmul(ns2w_ps[0:1, c:c + cw], ones16,
                                             sqw[:, c:c + cw],
                                             start=(f == 0), stop=(f == NF - 1))
                    # rsqrt free-major on one partition: exp(-0.5*ln(ns2+eps))
                    ns2w_fm = prep.tile([1, SCW], F32, tag="nsfm")
                    nc.scalar.activation(out=ns2w_fm[:, :scw], in_=ns2w_ps[0:1, :scw],
                                         func=AF.Ln, bias=epsb[0:1])
                    invw_fm16 = prep.tile([1, SCW], BF16, tag="invwfm")
                    nc.scalar.activation(out=invw_fm16[:, :scw], in_=ns2w_fm[:, :scw],
                                         func=AF.Exp, scale=-0.5)
                    nc.gpsimd.dma_start(out=wscr16[:, off:off + scw],
                                        in_=invw_fm16[:, :scw])
                    sl16 = wscr16[0:1, off:off + scw]
                    bc_src = bass.AP(tensor=sl16.tensor, offset=sl16.offset,
                                     ap=[[0, 128], [1, scw]])
                    nc.gpsimd.dma_start(out=invw16[:, off:off + scw], in_=bc_src)
                    # normalize + cast into the fp8 operand
                    sl = invw16[:, off:off + scw]
                    invw_b = bass.AP(tensor=sl.tensor, offset=sl.offset,
                                     ap=[sl.ap[0], [0, NF], sl.ap[1]])
                    nc.vector.tensor_tensor(out=wT8v[:, :, off:off + scw],
                                            in0=slab[:, :, :scw],
                                            in1=invw_b, op=ALU.mult)

                # ---------- main loop ----------
                # Per token-tile: one full-row p load + ln + p*log(p) reduce
                # (independent of the GEMM), and per 1024-wide superchunk the
                # G matmuls + fused exp/accum + p*l reduce (PSUM-gated, 4-deep).
                for j in range(NT if stage != "prep" else 0):
                    if stage != "mm":
                        p_sb = stream.tile([128, VP], BF16, tag="p", bufs=2)
                        nc.sync.dma_start(out=p_sb,
                                          in_=p_d[j * 128:(j + 1) * 128, :])

                    for s, (off, scw) in enumerate(SCS):
                        k = j * NSC + s
                        G = psum.tile([128, SCW], F32, tag="G")
                        for t in range(NF2):
                            for c in range(0, scw, 512):
                                cw = min(512, scw - c)
                                nc.tensor.matmul(
                                    G[:, c:c + cw],
                                    hT_sb[:, t, :, j * 128:(j + 1) * 128],
                                    wT_sb[:, t, :, off + c:off + c + cw],
                                    start=(t == 0), stop=(t == NF2 - 1),
                                    perf_mode=mybir.MatmulPerfMode.DoubleRow,
                                )
                        if stage == "mm":
                            continue

                        scr_e = scratch.tile([128, SCW], BF16, tag="scre")
                        nc.scalar.activation(out=scr_e[:, :scw], in_=G[:, :scw],
                                             func=AF.Exp, bias=neg1,
                                             scale=inv_s[:, j:j + 1],
                                             accum_out=zparts[:, k:k + 1])
                        if stage == "nodve":
                            continue

                        # bparts = sum_v (G*inv_s)*p = sum_v p*l
                        scr_b = scratch.tile([128, SCW], BF16, tag="scrb")
                        nc.vector.affine_mul_reduce(
                            out=scr_b[:, :scw], accum_out=bparts[:, k:k + 1],
                            in0=G[:, :scw], in1=p_sb[:, off:off + scw],
                            scale=inv_s[:, j:j + 1], bias=0.0)

                        # p*log(p) path is G-independent: two 3200-wide
                        # ln + reduce pairs per t-tile, spread mid-tile
                        if stage != "mm" and s in (2, 5):
                            ho = (VP // 2) * (s == 5)
                            hw = VP // 2
                            logp = scratch.tile([128, VP // 2], F32, tag="logp")
                            nc.scalar.activation(out=logp,
                                                 in_=p_sb[:, ho:ho + hw], func=AF.Ln)
                            # aparts = sum_v p*log(p); in-place overwrite of logp
                            nc.vector.affine_mul_reduce(
                                out=logp, accum_out=aparts[:, k:k + 1],
                                in0=p_sb[:, ho:ho + hw], in1=logp,
                                scale=1.0, bias=0.0)

                # collapse superchunk partials and write out
                if stage == "full":
                    zpm = persist.tile([128, NT], F32)
                    apm = persist.tile([128, NT], F32)
                    bpm = persist.tile([128, NT], F32)
                    for (src, dst) in ((zparts, zpm), (aparts, apm), (bparts, bpm)):
                        nc.vector.tensor_reduce(
                            out=dst, in_=src.rearrange("pp (j s) -> pp j s", s=NSC),
                            axis=mybir.AxisListType.X, op=ALU.add)
                    nc.sync.dma_start(out=z_d, in_=zpm)
                    nc.sync.dma_start(out=a_d, in_=apm)
                    nc.sync.dma_start(out=b_d, in_=bpm)

    nc.compile()
    return nc


def _get_program():
    if "nc" not in _CACHE:
        _CACHE["nc"] = _build_program()
    return _CACHE["nc"]


def _prep_inputs(h_student, W_vocab, p_teacher):
    """Host-side shard/layout prep (numpy only)."""
    TOK = B * L
    sp_s = np.ascontiguousarray(h_student.reshape(TOK, N_FEAT + 1)[:, 1:])
    sp_w = W_vocab[:, 1:]

    hT8 = np.ascontiguousarray(sp_s.astype(ml_dtypes.float8_e4m3).T)

    # padded W: zeros beyond V
    wn16_full = np.zeros((V_PAD_TOTAL, N_FEAT), dtype=ml_dtypes.bfloat16)
    wn16_full[:V] = sp_w.astype(ml_dtypes.bfloat16)
    wT16_full = np.ascontiguousarray(wn16_full.T)

    # padded p: ones beyond V (log 1 = 0, and padded G columns are 0)
    p16_full = np.ones((TOK, V_PAD_TOTAL), dtype=ml_dtypes.bfloat16)
    p16_full[:, :V] = p_teacher.reshape(TOK, V).astype(ml_dtypes.bfloat16)

    in_maps = []
    for k in range(N_CORES):
        lo, hi = k * VP, (k + 1) * VP
        in_maps.append({
            "hT": hT8,
            "wT": np.ascontiguousarray(wT16_full[:, lo:hi]),
            "p": np.ascontiguousarray(p16_full[:, lo:hi]),
        })
    return in_maps


def _combine(results, h_student, teacher_entropy):
    """Host-side gather of per-core row partials + tiny radial part."""
    TOK = B * L

    def pm_to_tok(arr):  # [128, NT] partition-major -> [TOK] token order
        return np.ascontiguousarray(arr.T).reshape(TOK)

    Z = np.zeros(TOK, np.float64)
    A = np.zeros(TOK, np.float64)
    Bp = np.zeros(TOK, np.float64)
    for k in range(N_CORES):
        Z += pm_to_tok(results[k]["Z"]).astype(np.float64)
        A += pm_to_tok(results[k]["A"]).astype(np.float64)
        Bp += pm_to_tok(results[k]["Bt"]).astype(np.float64)
    inv_s = pm_to_tok(results[0]["IS"]).astype(np.float64)

    # remove the padded columns' exp(0*inv_s - 1) contribution (core 7)
    Z -= N_PAD_LAST * math.exp(-1.0)

    logZ = 1.0 + np.log(Z)
    kl_rows = A - Bp + logZ  # Bp already inv_s-scaled on device
    kl = kl_rows.sum() / TOK
    l_angular = kl * (T_TEMP ** 2)

    # radial part from the raw fp32 inputs (O(B*L) work)
    x0 = np.clip(h_student.reshape(TOK, N_FEAT + 1)[:, 0].astype(np.float64),
                 1.0 + 1e-7, None)
    r_s = np.arccosh(x0)
    H_norm = np.clip(teacher_entropy.reshape(TOK).astype(np.float64) / LOG_V, 0.0, 1.0)
    r_target = (1.0 / (1.0 + np.exp(H_norm))) * R_MAX  # sigmoid(-H) * R_MAX
    l_radial = np.mean((r_s - r_target) ** 2)
    l_total = l_angular + LAMBDA_RADIAL * l_radial

    return np.array([l_total, l_angular, l_radial,
                     r_s.mean(), r_target.mean(), H_norm.mean()], dtype=np.float32)


def kernel(h_student, W_vocab, p_teacher, teacher_entropy):
    nc = _get_program()
    in_maps = _prep_inputs(h_student, W_vocab, p_teacher)
    res = bass_utils.run_bass_kernel_spmd(nc, in_maps, core_ids=list(range(N_CORES)))
    return _combine(res.results, h_student, teacher_entropy)


# revision 44
# speedup vs baseline: 2.5859x; 1.0580x over previous
"""Trainium2 Bass kernel for DecoupledRadialAngularLoss.

Strategy (vocab-parallel over 8 NeuronCores, like fused-linear-CE):
  - V=50257 padded to 51200 = 8*6400; core k owns vocab slice [k*6400,(k+1)*6400)
    (zero-padded W columns / p=1.0 padded teacher entries; exact host-side
    correction of the padded exp contribution).
  - Each core:
      * normalizes its W_vocab shard spatial rows (norms via ACT Square+accum
        in natural layout, rsqrt, broadcast, in-place column scale of the
        feature-major bf16 copy used by the PE),
      * computes student spatial norms the same way -> per-token inv_s,
      * GEMM G[t,v] = sp_s . u_w (bf16, fp32 PSUM accumulation),
      * ACT: exp(G*inv_s - 1) with fused per-row accumulation -> partial Z
        (cos<=1 so the fixed shift 1.0 replaces the softmax max pass),
      * ACT: log(p); DVE tensor_tensor_reduce: partial sum p*log(p) and
        partial sum p*G per row.
  - Host combines per-core row partials: logZ = 1 + log(sum_k Z_k),
    KL_row = A - inv_s*B + logZ; radial loss terms are O(B*L) and computed
    on host from the raw fp32 inputs.
"""

import math

import ml_dtypes
import numpy as np

import concourse.bass as bass
import concourse.mybir as mybir
import concourse.tile as tile
from concourse import bacc
from concourse import bass_utils

# ---- problem constants (hardcoded per contest contract) ----
B, L, N_FEAT = 2, 1024, 768
V = 50257
R_MAX = 3.0
LAMBDA_RADIAL = 0.1
T_TEMP = 1.0
LOG_V = math.log(V)
EPS = 1e-12

N_CORES = 8
VP = 6400                 # per-core padded vocab shard (50*128, 12.5*512)
V_PAD_TOTAL = N_CORES * VP  # 51200
N_PAD_LAST = V_PAD_TOTAL - V  # 943 zero-W / one-p padded columns on core 7

NT = (B * L) // 128       # 16 token tiles of 128
NF = N_FEAT // 128        # 6 feature tiles of 128
SCW = 1024                # superchunk width (2 PSUM banks; 4-deep pipeline)
SCS = [(o, min(SCW, VP - o)) for o in range(0, VP, SCW)]  # 6x1024 + 256
NSC = len(SCS)

BF16 = mybir.dt.bfloat16
FP8 = mybir.dt.float8e4
F32 = mybir.dt.float32
AF = mybir.ActivationFunctionType
ALU = mybir.AluOpType
NF2 = NF // 2             # 3 DoubleRow k-tile pairs

_CACHE = {}


def _patch_act_tables():
    """Make Exp and Ln resolve to the one table set containing both
    (natural_log_exp_and_others) so the kernel's alternating exp/ln
    activations don't thrash ACT table loads (~1.3us each)."""
    if _CACHE.get("act_patched"):
        return
    from concourse import bacc as bacc_mod
    orig = bacc_mod.get_activation_tables

    def patched(arch):
        tabs = {k: set(v) for k, v in orig(arch).items()}
        for name in ("exp_and_others", "exp_and_friends"):
            if name in tabs:
                tabs[name].discard(AF.Exp)
        if "natural_log" in tabs:
            tabs["natural_log"].discard(AF.Ln)
        return tabs

    bacc_mod.get_activation_tables = patched
    _CACHE["act_patched"] = True


def _build_program(stage="full"):
    """Build + compile the single-core SPMD Bass program (same NEFF, 8 cores).

    stage: debug knob - "prep" builds only the norm/scale prep, "mm" adds
    the matmuls, "nodve" adds ACT exp/ln, "full" is everything.
    """
    _patch_act_tables()
    nc = bacc.Bacc("TRN2", target_bir_lowering=False, debug=False)

    TOK = B * L
    hT_d = nc.dram_tensor("hT", (N_FEAT, TOK), FP8, kind="ExternalInput").ap()
    wT_d = nc.dram_tensor("wT", (N_FEAT, VP), BF16, kind="ExternalInput").ap()
    p_d = nc.dram_tensor("p", (TOK, VP), BF16, kind="ExternalInput").ap()

    z_d = nc.dram_tensor("Z", (128, NT), F32, kind="ExternalOutput").ap()
    a_d = nc.dram_tensor("A", (128, NT), F32, kind="ExternalOutput").ap()
    b_d = nc.dram_tensor("Bt", (128, NT), F32, kind="ExternalOutput").ap()
    is_d = nc.dram_tensor("IS", (128, NT), F32, kind="ExternalOutput").ap()

    NWV = VP // 128  # 50 vocab norm tiles

    with tile.TileContext(nc) as tc:
        with (
            tc.tile_pool(name="persist", bufs=1) as persist,
            tc.tile_pool(name="dram", bufs=1, space="DRAM") as dram,
        ):
            # ---------- resident tiles ----------
            # fp8 GEMM operands in DoubleRow layout [k, t, q, x]
            # (feature f = (t*2+q)*128 + k)
            hT_sb = persist.tile([128, NF2, 2, TOK], FP8)
            wT_sb = persist.tile([128, NF2, 2, VP], FP8)
            invw16 = persist.tile([128, VP], BF16)
            inv_s = persist.tile([128, NT], F32)
            neg1 = persist.tile([128, 1], F32)
            epsb = persist.tile([128, 1], F32)
            zparts = persist.tile([128, NT * NSC], F32)
            aparts = persist.tile([128, NT * NSC], F32)
            bparts = persist.tile([128, NT * NSC], F32)

            nc.vector.memset(neg1, -1.0)
            nc.vector.memset(epsb, 1e-30)
            # aparts only uses slot 0 of each t-tile group; zero the rest
            nc.vector.memset(aparts, 0.0)

            with (
                tc.tile_pool(name="prep", bufs=2) as prep,
                tc.tile_pool(name="stream", bufs=3) as stream,
                tc.tile_pool(name="scratch", bufs=2) as scratch,
                tc.tile_pool(name="psum", bufs=4, space="PSUM") as psum,
            ):
                ones16 = persist.tile([128, 1], BF16)
                nc.vector.memset(ones16, 1.0)

                # fp8 student operand load (DoubleRow view of natural layout)
                nc.sync.dma_start(
                    out=hT_sb,
                    in_=hT_d.rearrange("(t q pp) x -> pp t q x", pp=128, q=2))

                # ---- student norms from hT8 (PE ones-matmul over partitions),
                #      rsqrt done partition-major after a DRAM roundtrip ----
                hscr = dram.tile([1, TOK], F32)
                for hc in range(0, TOK, SCW):
                    ns2h_ps = psum.tile([128, SCW], F32, tag="G")
                    for fq in range(NF):
                        sqh = prep.tile([128, SCW], BF16, tag="sq")
                        nc.scalar.activation(
                            out=sqh, in_=hT_sb[:, fq // 2, fq % 2, hc:hc + SCW],
                            func=AF.Square)
                        for c in range(0, SCW, 512):
                            nc.tensor.matmul(ns2h_ps[0:1, c:c + 512], ones16,
                                             sqh[:, c:c + 512],
                                             start=(fq == 0), stop=(fq == NF - 1))
                    ns2h_fm = prep.tile([1, SCW], F32, tag="nsfm")
                    nc.vector.tensor_copy(out=ns2h_fm, in_=ns2h_ps[0:1, :])
                    nc.sync.dma_start(out=hscr[:, hc:hc + SCW], in_=ns2h_fm)
                # rsqrt = exp(-0.5*ln(x)) - stays in the natural_log_exp
                # table set (no Sqrt set load/thrash)
                ns2_s = persist.tile([128, NT], F32)
                nc.sync.dma_start(out=ns2_s,
                                  in_=hscr.rearrange("one (j pp) -> one pp j", pp=128)[0])
                nc.scalar.activation(out=ns2_s, in_=ns2_s, func=AF.Ln, bias=epsb)
                nc.scalar.activation(out=inv_s, in_=ns2_s, func=AF.Exp, scale=-0.5)
                nc.sync.dma_start(out=is_d, in_=inv_s)

                # ---- vocab norms + rsqrt + column scale, per superchunk.
                # wT arrives as transient bf16 slabs; squares and the
                # normalize run at bf16 DVE rates, the normalized result is
                # written straight into the fp8 DoubleRow-layout operand. ----
                wscr16 = dram.tile([1, VP], BF16)   # flat inv_w (bf16)
                wT_r = wT_d.rearrange("(f pp) v -> pp f v", pp=128)
                # first W slab loads before hT8 so the DVE-critical chain
                # starts as early as possible (write precedes all readers)
                slab0 = prep.tile([128, NF, SCW], BF16, tag="slab", bufs=3)
                nc.sync.dma_start(out=slab0, in_=wT_r[:, :, 0:SCW])
                # [128, (t q), v] view of the fp8 operand; f-tile = t*2+q
                wT8v = wT_sb.rearrange("pp t q v -> pp (t q) v")
                for (off, scw) in SCS:
                    if off == 0:
                        slab = slab0
                    else:
                        slab = prep.tile([128, NF, SCW], BF16, tag="slab", bufs=3)
                        nc.sync.dma_start(out=slab[:, :, :scw],
                                          in_=wT_r[:, :, off:off + scw])
                    ns2w_ps = psum.tile([128, SCW], F32, tag="G")
                    for f in range(NF):
                        sqw = prep.tile([128, SCW], BF16, tag="sq")
                        # split squares across DVE and ACT to balance load
                        if f in (0, 2):
                            nc.vector.tensor_tensor(out=sqw[:, :scw],
                                                    in0=slab[:, f, :scw],
                                                    in1=slab[:, f, :scw],
                                                    op=ALU.mult)
                        else:
                            nc.scalar.activation(out=sqw[:, :scw],
                                                 in_=slab[:, f, :scw],
                                                 func=AF.Square)
                        for c in range(0, scw, 512):
                            cw = min(512, scw - c)
                            nc.tensor.matmul(ns2w_ps[0:1, c:c + cw], ones16,
                                             sqw[:, c:c + cw],
                                             start=(f == 0), stop=(f == NF - 1))
                    # rsqrt free-major on one partition: exp(-0.5*ln(ns2+eps))
                    ns2w_fm = prep.tile([1, SCW], F32, tag="nsfm")
                    nc.scalar.activation(out=ns2w_fm[:, :scw], in_=ns2w_ps[0:1, :scw],
                                         func=AF.Ln, bias=epsb[0:1])
                    invw_fm16 = prep.tile([1, SCW], BF16, tag="invwfm")
                    nc.scalar.activation(out=invw_fm16[:, :scw], in_=ns2w_fm[:, :scw],
                                         func=AF.Exp, scale=-0.5)
                    nc.gpsimd.dma_start(out=wscr16[:, off:off + scw],
                                        in_=invw_fm16[:, :scw])
                    sl16 = wscr16[0:1, off:off + scw]
                    bc_src = bass.AP(tensor=sl16.tensor, offset=sl16.offset,
                                     ap=[[0, 128], [1, scw]])
                    nc.gpsimd.dma_start(out=invw16[:, off:off + scw], in_=bc_src)
                    # normalize + cast into the fp8 operand
                    sl = invw16[:, off:off + scw]
                    invw_b = bass.AP(tensor=sl.tensor, offset=sl.offset,
                                     ap=[sl.ap[0], [0, NF], sl.ap[1]])
                    nc.vector.tensor_tensor(out=wT8v[:, :, off:off + scw],
                                            in0=slab[:, :, :scw],
                                            in1=invw_b, op=ALU.mult)

                # ---------- main loop ----------
                # Per token-tile: one full-row p load + ln + p*log(p) reduce
                # (independent of the GEMM), and per 1024-wide superchunk the
                # G matmuls + fused exp/accum + p*l reduce (PSUM-gated, 4-deep).
                for j in range(NT if stage != "prep" else 0):
                    if stage != "mm":
                        p_sb = stream.tile([128, VP], BF16, tag="p", bufs=2)
                        nc.sync.dma_start(out=p_sb,
                                          in_=p_d[j * 128:(j + 1) * 128, :])

                    for s, (off, scw) in enumerate(SCS):
                        k = j * NSC + s
                        G = psum.tile([128, SCW], F32, tag="G")
                        for t in range(NF2):
                            for c in range(0, scw, 512):
                                cw = min(512, scw - c)
                                nc.tensor.matmul(
                                    G[:, c:c + cw],
                                    hT_sb[:, t, :, j * 128:(j + 1) * 128],
                                    wT_sb[:, t, :, off + c:off + c + cw],
                                    start=(t == 0), stop=(t == NF2 - 1),
                                    perf_mode=mybir.MatmulPerfMode.DoubleRow,
                                )
                        if stage == "mm":
                            continue

                        scr_e = scratch.tile([128, SCW], BF16, tag="scre")
                        nc.scalar.activation(out=scr_e[:, :scw], in_=G[:, :scw],
                                             func=AF.Exp, bias=neg1,
                                             scale=inv_s[:, j:j + 1],
                                             accum_out=zparts[:, k:k + 1])
                        if stage == "nodve":
                            continue

                        # bparts = sum_v (G*inv_s)*p = sum_v p*l
                        scr_b = scratch.tile([128, SCW], BF16, tag="scrb")
                        nc.vector.affine_mul_reduce(
                            out=scr_b[:, :scw], accum_out=bparts[:, k:k + 1],
                            in0=G[:, :scw], in1=p_sb[:, off:off + scw],
                            scale=inv_s[:, j:j + 1], bias=0.0)

                        # p*log(p) path is G-independent: two 3200-wide
                        # ln + reduce pairs per t-tile, spread mid-tile
                        if stage != "mm" and s in (2, 5):
                            ho = (VP // 2) * (s == 5)
                            hw = VP // 2
                            logp = scratch.tile([128, VP // 2], F32, tag="logp")
                            nc.scalar.activation(out=logp,
                                                 in_=p_sb[:, ho:ho + hw], func=AF.Ln)
                            # aparts = sum_v p*log(p); in-place overwrite of logp
                            nc.vector.affine_mul_reduce(
                                out=logp, accum_out=aparts[:, k:k + 1],
                                in0=p_sb[:, ho:ho + hw], in1=logp,
                                scale=1.0, bias=0.0)

                # collapse superchunk partials and write out
                if stage == "full":
                    zpm = persist.tile([128, NT], F32)
                    apm = persist.tile([128, NT], F32)
                    bpm = persist.tile([128, NT], F32)
                    for (src, dst) in ((zparts, zpm), (aparts, apm), (bparts, bpm)):
                        nc.vector.tensor_reduce(
                            out=dst, in_=src.rearrange("pp (j s) -> pp j s", s=NSC),
                            axis=mybir.AxisListType.X, op=ALU.add)
                    nc.sync.dma_start(out=z_d, in_=zpm)
                    nc.sync.dma_start(out=a_d, in_=apm)
                    nc.sync.dma_start(out=b_d, in_=bpm)

    nc.compile()
    return nc


def _get_program():
    if "nc" not in _CACHE:
        _CACHE["nc"] = _build_program()
    return _CACHE["nc"]


def _prep_inputs(h_student, W_vocab, p_teacher):
    """Host-side shard/layout prep (numpy only)."""
    TOK = B * L
    sp_s = np.ascontiguousarray(h_student.reshape(TOK, N_FEAT + 1)[:, 1:])
    sp_w = W_vocab[:, 1:]

    hT8 = np.ascontiguousarray(sp_s.astype(ml_dtypes.float8_e4m3).T)

    # padded W: zeros beyond V
    wn16_full = np.zeros((V_PAD_TOTAL, N_FEAT), dtype=ml_dtypes.bfloat16)
    wn16_full[:V] = sp_w.astype(ml_dtypes.bfloat16)
    wT16_full = np.ascontiguousarray(wn16_full.T)

    # padded p: ones beyond V (log 1 = 0, and padded G columns are 0)
    p16_full = np.ones((TOK, V_PAD_TOTAL), dtype=ml_dtypes.bfloat16)
    p16_full[:, :V] = p_teacher.reshape(TOK, V).astype(ml_dtypes.bfloat16)

    in_maps = []
    for k in range(N_CORES):
        lo, hi = k * VP, (k + 1) * VP
        in_maps.append({
            "hT": hT8,
            "wT": np.ascontiguousarray(wT16_full[:, lo:hi]),
            "p": np.ascontiguousarray(p16_full[:, lo:hi]),
        })
    return in_maps


def _combine(results, h_student, teacher_entropy):
    """Host-side gather of per-core row partials + tiny radial part."""
    TOK = B * L

    def pm_to_tok(arr):  # [128, NT] partition-major -> [TOK] token order
        return np.ascontiguousarray(arr.T).reshape(TOK)

    Z = np.zeros(TOK, np.float64)
    A = np.zeros(TOK, np.float64)
    Bp = np.zeros(TOK, np.float64)
    for k in range(N_CORES):
        Z += pm_to_tok(results[k]["Z"]).astype(np.float64)
        A += pm_to_tok(results[k]["A"]).astype(np.float64)
        Bp += pm_to_tok(results[k]["Bt"]).astype(np.float64)
    inv_s = pm_to_tok(results[0]["IS"]).astype(np.float64)

    # remove the padded columns' exp(0*inv_s - 1) contribution (core 7)
    Z -= N_PAD_LAST * math.exp(-1.0)

    logZ = 1.0 + np.log(Z)
    kl_rows = A - Bp + logZ  # Bp already inv_s-scaled on device
    kl = kl_rows.sum() / TOK
    l_angular = kl * (T_TEMP ** 2)

    # radial part from the raw fp32 inputs (O(B*L) work)
    x0 = np.clip(h_student.reshape(TOK, N_FEAT + 1)[:, 0].astype(np.float64),
                 1.0 + 1e-7, None)
    r_s = np.arccosh(x0)
    H_norm = np.clip(teacher_entropy.reshape(TOK).astype(np.float64) / LOG_V, 0.0, 1.0)
    r_target = (1.0 / (1.0 + np.exp(H_norm))) * R_MAX  # sigmoid(-H) * R_MAX
    l_radial = np.mean((r_s - r_target) ** 2)
    l_total = l_angular + LAMBDA_RADIAL * l_radial

    return np.array([l_total, l_angular, l_radial,
                     r_s.mean(), r_target.mean(), H_norm.mean()], dtype=np.float32)


def kernel(h_student, W_vocab, p_teacher, teacher_entropy):
    nc = _get_program()
    in_maps = _prep_inputs(h_student, W_vocab, p_teacher)
    res = bass_utils.run_bass_kernel_spmd(nc, in_maps, core_ids=list(range(N_CORES)))
    return _combine(res.results, h_student, teacher_entropy)
